# revision 4
# baseline (speedup 1.0000x reference)
"""nn_BlockMasking — 8-core Trainium2 Bass kernel.

Pipeline (pure data-parallel over the batch dim, 4 samples/core):
  launch 1 (Bass, 8 cores): stream the 65.5MB/core input, per-patch max|x|
      reduce (ACT abs -> bf16, DVE max trees, DRAM bounce for the
      partition-crossing (ds,hs) reduction) -> validmax [4,10,10,10]/core.
  host: valid = validmax > 0; counts/len_keep/targets; the reference's
      per-sample rejection sampler runs verbatim under jax on CPU (tiny
      [32,1000] state; bit-exact vs the reference's RNG stream).
  launch 2 (Bass, 8 cores): expand per-patch visible flags to the full
      [4,160,160,160] f32 output/core via step-0-broadcast tensor_copy +
      dense 1.6MB DMAs (65.5MB/core written).

Both launches are HBM-bandwidth-bound (~183us/core each at 358GB/s).
"""
import time

import numpy as np

import concourse.bacc as bacc
import concourse.mybir as mybir
import concourse.tile as tile
from concourse import bass_utils

F32 = mybir.dt.float32
BF16 = mybir.dt.bfloat16

# ---- module config (hardcoded; matches the nn_BlockMasking problem) ----
MASK_RATIO = 0.75
IMG = (160, 160, 160)
PATCH = (16, 16, 16)
GRID = (10, 10, 10)
BLOCK = (2, 2, 2)
MAX_ATTEMPTS = 1000
MAX_STARTS = (9, 9, 9)
N_PATCH = 1000
B = 32
N_CORES = 8
BPC = B // N_CORES  # samples per core
N_SLAB = BPC * GRID[0]  # 40 output slabs per core (one per (sample, d-patch))

# Timing info for the most recent kernel() call (filled when BASS_TRACE=1):
LAST_EXEC_TIMES_NS = []


# ---------------------------------------------------------------------------
# Verbatim copy of the reference's sampler (must trace identically so the
# RNG stream matches bit-for-bit).
# ---------------------------------------------------------------------------
def _sample_block_mask(valid, target, key):
    import jax
    import jax.numpy as jnp

    max_starts = jnp.array(MAX_STARTS, dtype=jnp.int32)

    def body(i, carry):
        masked, count, key = carry
        key, k1 = jax.random.split(key)
        starts = jax.random.randint(k1, (3,), 0, max_starts)
        s = (starts[0], starts[1], starts[2])
        avail_block = jax.lax.dynamic_slice(valid & ~masked, s, BLOCK)
        num_new = jnp.sum(avail_block).astype(jnp.int32)
        remaining = target - count
        accept = (num_new > 0) & (num_new <= remaining)
        cur_block = jax.lax.dynamic_slice(masked, s, BLOCK)
        new_block = jnp.where(accept, cur_block | avail_block, cur_block)
        masked = jax.lax.dynamic_update_slice(masked, new_block, s)
        count = count + jnp.where(accept, num_new, jnp.int32(0))
        return masked, count, key

    masked0 = jnp.zeros_like(valid)
    masked, count, key = jax.lax.fori_loop(
        0, MAX_ATTEMPTS, body, (masked0, jnp.int32(0), key))

    remaining = target - count
    key, k2 = jax.random.split(key)
    scores = jax.random.uniform(k2, (N_PATCH,), dtype=jnp.float32)
    avail = (valid & ~masked).ravel()
    scores = jnp.where(avail, scores, jnp.inf)
    rank = jnp.argsort(jnp.argsort(scores))
    fill = avail & (rank < remaining)
    return masked | fill.reshape(valid.shape)


# ---------------------------------------------------------------------------
# Launch 1: per-patch max|x| reduce.
# ---------------------------------------------------------------------------
def build_reduce_kernel():
    nc = bacc.Bacc("TRN2", target_bir_lowering=False, debug=False)
    img = nc.dram_tensor("img", [BPC, 160, 160, 160], F32, kind="ExternalInput")
    vout = nc.dram_tensor("validmax", [BPC, 100, 10], F32, kind="ExternalOutput")
    bounce = nc.dram_tensor("bounce", [25, 128, 32, 10], BF16, kind="ExternalOutput")

    MAX = mybir.AluOpType.max
    img_t = img.ap().rearrange("b d h w -> (b d h w)").rearrange(
        "(t p f) -> t p f", t=25, p=128, f=5120)
    bounce_flat = bounce.ap().rearrange("t p j w -> (t p j w)")
    # bounce flat idx = g*10+wp with g = (b,d,h) row index
    bounce_src = bounce_flat.rearrange(
        "(b dp ds hp hswp) -> b ds dp hp hswp", b=BPC, dp=10, ds=16, hp=10, hswp=160)

    with tile.TileContext(nc) as tc:
        with (
            tc.tile_pool(name="inp", bufs=3) as inp_pool,
            tc.tile_pool(name="scr", bufs=2) as scr_pool,
            tc.tile_pool(name="rm", bufs=1) as rm_pool,
            tc.tile_pool(name="sb", bufs=2) as sb_pool,
            tc.tile_pool(name="sc", bufs=2) as sc_pool,
        ):
            rm_all = rm_pool.tile([128, 8000], BF16)
            for t in range(25):
                v = inp_pool.tile([128, 5120], F32)
                nc.sync.dma_start(v[:], img_t[t])
                va = scr_pool.tile([128, 5120], BF16, tag="vabs")
                nc.scalar.activation(va[:], v[:], mybir.ActivationFunctionType.Abs)
                v4 = va[:].rearrange("p (r wp ws) -> p r wp ws", r=32, wp=10, ws=16)
                t1 = scr_pool.tile([128, 2560], BF16, tag="t1")
                t1v = t1[:].rearrange("p (r wp ws) -> p r wp ws", r=32, wp=10, ws=8)
                nc.vector.tensor_tensor(t1v, v4[:, :, :, 0:8], v4[:, :, :, 8:16], MAX)
                t2 = scr_pool.tile([128, 1280], BF16, tag="t2")
                t2v = t2[:].rearrange("p (r wp ws) -> p r wp ws", r=32, wp=10, ws=4)
                nc.vector.tensor_tensor(t2v, t1v[:, :, :, 0:4], t1v[:, :, :, 4:8], MAX)
                t3 = scr_pool.tile([128, 640], BF16, tag="t3")
                t3v = t3[:].rearrange("p (r wp ws) -> p r wp ws", r=32, wp=10, ws=2)
                nc.vector.tensor_tensor(t3v, t2v[:, :, :, 0:2], t2v[:, :, :, 2:4], MAX)
                rmv = rm_all[:, t * 320:(t + 1) * 320].rearrange(
                    "p (r wp) -> p r wp", r=32, wp=10)
                nc.vector.tensor_tensor(rmv, t3v[:, :, :, 0], t3v[:, :, :, 1], MAX)
            rm_view = rm_all[:].rearrange("p (t jw) -> p t jw", t=25, jw=320)
            bounce_dst = bounce.ap().rearrange("t p j w -> p t (j w)")
            nc.sync.dma_start(bounce_dst, rm_view)

            for b in range(BPC):
                tb = sb_pool.tile([100, 2560], BF16)
                tbv = tb[:].rearrange("q (ds hs wp) -> q ds hs wp", ds=16, hs=16, wp=10)
                for ds in range(16):
                    nc.sync.dma_start(tb[:, ds * 160:(ds + 1) * 160], bounce_src[b, ds])
                cur = tbv
                for i, dsn in enumerate([8, 4, 2, 1]):
                    nt = sc_pool.tile([100, dsn * 160], BF16, tag=f"u{i}")
                    ntv = nt[:].rearrange("q (ds hs wp) -> q ds hs wp",
                                          ds=dsn, hs=16, wp=10)
                    nc.vector.tensor_tensor(ntv, cur[:, 0:dsn], cur[:, dsn:2 * dsn], MAX)
                    cur = ntv
                cur = cur[:, 0]
                for i, hsn in enumerate([8, 4, 2, 1]):
                    nt = sc_pool.tile([100, hsn * 10], BF16, tag=f"w{i}")
                    ntv = nt[:].rearrange("q (hs wp) -> q hs wp", hs=hsn, wp=10)
                    nc.vector.tensor_tensor(ntv, cur[:, 0:hsn], cur[:, hsn:2 * hsn], MAX)
                    cur = ntv
                wf = sc_pool.tile([100, 10], F32, tag="wf")
                nc.vector.tensor_copy(wf[:], cur[:, 0])
                nc.sync.dma_start(vout.ap()[b], wf[:])
    nc.compile()
    return nc


# ---------------------------------------------------------------------------
# Launch 2: expand visible flags to voxel output.
# ---------------------------------------------------------------------------
def build_bcast_kernel():
    nc = bacc.Bacc("TRN2", target_bir_lowering=False, debug=False)
    flags = nc.dram_tensor("flags", [N_SLAB, 128, 200], F32, kind="ExternalInput")
    out = nc.dram_tensor("out", [BPC, 160, 160, 160], F32, kind="ExternalOutput")
    out_v = out.ap().rearrange("b d h w -> (b d h w)").rearrange(
        "(s p f) -> s p f", s=N_SLAB, p=128, f=3200)

    with tile.TileContext(nc) as tc:
        with (
            tc.tile_pool(name="fl", bufs=4) as fl_pool,
            tc.tile_pool(name="slab", bufs=4) as slab_pool,
        ):
            for s in range(N_SLAB):
                fl = fl_pool.tile([128, 200], F32)
                nc.sync.dma_start(fl[:], flags.ap()[s])
                slab = slab_pool.tile([128, 3200], F32)
                src = fl[:].rearrange("p (k wp) -> p k wp", k=20, wp=10)
                src = src.unsqueeze(3).broadcast_to([128, 20, 10, 16])
                dst = slab[:].rearrange("p (k wp ws) -> p k wp ws", k=20, wp=10, ws=16)
                nc.vector.tensor_copy(dst, src)
                nc.sync.dma_start(out_v[s], slab[:])
    nc.compile()
    return nc


_CACHE = {}


def _get(name, builder):
    if name not in _CACHE:
        _CACHE[name] = builder()
    return _CACHE[name]


def _flags200_from_visible(visible):
    """visible [32,10,10,10] bool -> [8, 40, 128, 200] f32 per-core flag tiles.

    Output slab s=(b_local, dc) row layout: partition p holds 20 rows
    (rows 20p..20p+19 of the [2560,160] slab); row k of partition p is the
    h-row hp=(20*(p%8)+k)//16, whose 160 voxels are visible[b,dc,hp,wp]
    expanded 16x along ws.
    """
    p = np.arange(128)
    k = np.arange(20)
    hp_idx = (20 * (p % 8)[:, None] + k[None, :]) // 16  # [128,20]
    arr = visible[:, :, hp_idx, :]  # [32, 10, 128, 20, 10]
    return np.ascontiguousarray(
        arr.reshape(N_CORES, N_SLAB, 128, 200).astype(np.float32))


def kernel(img_mask):
    global LAST_EXEC_TIMES_NS
    LAST_EXEC_TIMES_NS = []
    in_dtype = img_mask.dtype
    img = np.ascontiguousarray(np.asarray(img_mask, dtype=np.float32))
    assert img.shape == (B, 160, 160, 160)

    # ---- launch 1: per-patch max|x| on the 8 cores ----
    nc1 = _get("reduce", build_reduce_kernel)
    in_maps1 = [{"img": img[c * BPC:(c + 1) * BPC]} for c in range(N_CORES)]
    t0 = time.time()
    res1 = bass_utils.run_bass_kernel_spmd(nc1, in_maps1, core_ids=list(range(N_CORES)))
    LAST_EXEC_TIMES_NS.append(res1.exec_time_ns or int((time.time() - t0) * 1e9))
    validmax = np.concatenate(
        [np.asarray(res1.results[c]["validmax"]).reshape(BPC, 10, 10, 10)
         for c in range(N_CORES)], axis=0)

    # ---- host: reference-identical mask sampling (tiny state, CPU jax) ----
    import jax
    import jax.numpy as jnp
    valid = validmax > 0  # [32,10,10,10] bool
    counts = valid.reshape(B, -1).sum(axis=1).astype(np.int32)
    len_keep = np.int32(np.floor((1.0 - MASK_RATIO) * np.float32(counts.min())))
    targets = (counts - len_keep).astype(np.int32)
    cpu = jax.devices("cpu")[0]
    with jax.default_device(cpu):
        keys = jax.random.split(jax.random.key(42), B)
        masked = np.asarray(jax.vmap(_sample_block_mask)(
            jnp.asarray(valid), jnp.asarray(targets), keys))
    visible = valid & ~masked

    # ---- launch 2: broadcast-expand to the voxel mask ----
    nc2 = _get("bcast", build_bcast_kernel)
    flags = _flags200_from_visible(visible)
    in_maps2 = [{"flags": flags[c]} for c in range(N_CORES)]
    t0 = time.time()
    res2 = bass_utils.run_bass_kernel_spmd(nc2, in_maps2, core_ids=list(range(N_CORES)))
    LAST_EXEC_TIMES_NS.append(res2.exec_time_ns or int((time.time() - t0) * 1e9))
    out = np.concatenate(
        [np.asarray(res2.results[c]["out"]) for c in range(N_CORES)], axis=0)
    return out.astype(in_dtype, copy=False)
